# revision 43
# baseline (speedup 1.0000x reference)
"""Trainium2 Bass kernel for the cross-attention transformer block.

Strategy (8 NeuronCores, data-parallel over batch B=8, one batch item per core):

  - LN1/LN2 are FOLDED INTO the QKV projections: q = rstd*(Wq'^T x) -
    (mu*rstd)*colsum(Wq') (+ beta terms), with gamma folded into the weights
    host-side.  The QKV matmuls therefore consume the raw bf16 x/y straight
    from DMA, and the LN stats (ones-vector matmuls) interleave with them so
    the PE array stays dense and warm from the start.
  - rstd is computed as exp(-0.5*ln(var+eps)) so that ONE activation table
    set (natural_log_exp_and_others: exp/ln/square) covers layernorms AND the
    attention softmax; the only table switch in the whole kernel is to gelu.
  - Attention: scores^T = K^T(d,m)Q(d,n) with row-packed (tile_position) pairs
    of heads; softmax skips the max-subtraction (|s|<6 for these shapes); exp
    runs on ACT straight out of PSUM in [128,1024] chunks, double-buffered.
    V carries an appended ones column per head, and the AV matmuls are
    column-packed 2 heads per 128-wide PSUM tile, so each e-tile feeds two
    concurrent col-tiled matmuls.  W1 is host-permuted (with zero rows) to
    match the packed o layout.
  - The token dim is split into two 512-column blocks A/B and software-
    pipelined: FFN(A) (PE-heavy) runs WHILE attention(B)'s exp stream
    (ACT-heavy) executes; gelu(A) is batched after the last exp(B) to avoid
    activation-table thrash.
  - FFN weights (16 MB bf16) are streamed from HBM once per block.
"""

import sys

for _p in ("/opt/trn_rl_repo", "/root/.axon_site/_ro/trn_rl_repo"):
    if _p not in sys.path:
        sys.path.append(_p)

from contextlib import ExitStack

import numpy as np
import ml_dtypes

import concourse.bacc as bacc
import concourse.mybir as mybir
from concourse.tile import TileContext
from concourse import bass_utils

F32 = mybir.dt.float32
BF16 = mybir.dt.bfloat16
AF = mybir.ActivationFunctionType
OP = mybir.AluOpType

P = 128
B, N, C, H, D, W = 8, 1024, 1024, 16, 32, 4
HD = H * D            # 512
DA = 2 * D            # 64: V + ones column (padded)
F = W * C             # 4096
KT = C // P           # 8 feature k-tiles
EPS = 1e-5
NCORES = 8

_BUILD_CACHE = {}
_LAST_IN_MAPS = None


def _build(flags):
    """flags = (f_a1, f_bq, f_bkv, f_b1, f_b2, f_b3)"""
    f_a1, f_bq, f_bkv, f_b1, f_b2, f_b3 = flags
    nc = bacc.Bacc("TRN2", target_bir_lowering=False)

    xb_d = nc.dram_tensor("xb", [C, N], BF16, kind="ExternalInput")
    yb_d = nc.dram_tensor("yb", [C, N], BF16, kind="ExternalInput")
    wq_d = nc.dram_tensor("wq", [C, HD], BF16, kind="ExternalInput")
    wk_d = nc.dram_tensor("wk", [C, HD], BF16, kind="ExternalInput")
    wv_d = nc.dram_tensor("wv", [C, HD], BF16, kind="ExternalInput")
    aq_d = nc.dram_tensor("aq", [HD, 1], F32, kind="ExternalInput")
    ak_d = nc.dram_tensor("ak", [HD, 1], F32, kind="ExternalInput")
    av_d = nc.dram_tensor("av", [1, HD], F32, kind="ExternalInput")
    w1p_d = nc.dram_tensor("w1p", [2 * HD, C], BF16, kind="ExternalInput")
    w2_d = nc.dram_tensor("w2", [C, F], BF16, kind="ExternalInput")
    w3_d = nc.dram_tensor("w3", [F, C], BF16, kind="ExternalInput")
    vecs = {}
    if f_bq:
        vecs["bq"] = nc.dram_tensor("bq", [HD, 1], F32, kind="ExternalInput")
    if f_bkv:
        vecs["bk"] = nc.dram_tensor("bk", [HD, 1], F32, kind="ExternalInput")
        vecs["bv"] = nc.dram_tensor("bv", [1, HD], F32, kind="ExternalInput")
    if f_a1:
        vecs["g1"] = nc.dram_tensor("g1", [C, 1], F32, kind="ExternalInput")
        vecs["be1"] = nc.dram_tensor("be1", [C, 1], F32, kind="ExternalInput")
    if f_b1:
        vecs["b1"] = nc.dram_tensor("b1", [C, 1], F32, kind="ExternalInput")
    if f_b2:
        vecs["b2"] = nc.dram_tensor("b2", [F, 1], F32, kind="ExternalInput")
    if f_b3:
        vecs["b3"] = nc.dram_tensor("b3", [C, 1], F32, kind="ExternalInput")
    OT = nc.dram_tensor("OT", [C, N], F32, kind="ExternalOutput")

    with TileContext(nc) as tc:
        with ExitStack() as top_es:
            def pool(name, bufs, space=None):
                kw = {"space": space} if space else {}
                return top_es.enter_context(
                    tc.tile_pool(name=name, bufs=bufs, **kw))

            p_const = pool("p_const", 1)
            p_x = pool("p_x", 1)
            p_qk = pool("p_qk", 1)
            p_v = pool("p_v", 1)
            p_bc = pool("p_bc", 1)
            p_out1 = pool("p_out1", 16)
            p_park = pool("p_park", 32)
            p_tmp = pool("p_tmp", 4)
            p_ln3 = pool("p_ln3", 8)
            # ---------------- constants ----------------
            ones_col = p_const.tile([P, 1], BF16, name="ones_col")
            nc.vector.memset(ones_col, 1.0)
            ones_row = p_const.tile([1, P], F32, name="ones_row")
            nc.vector.memset(ones_row, 1.0)
            ones1 = p_const.tile([1, 1], F32, name="ones1")
            nc.vector.memset(ones1, 1.0)
            eps1 = p_const.tile([1, 1], F32, name="eps1")
            nc.vector.memset(eps1, EPS)
            # E2[k,p] = 1 iff (k==32 and p<64) or (k==96 and p>=64):
            # broadcasts the per-head Z rows of a packed aug tile
            e2 = p_const.tile([P, P], F32, name="e2")
            nc.vector.memset(e2, 0.0)
            nc.vector.memset(e2[32:33, 0:64], 1.0)
            nc.vector.memset(e2[96:97, 64:128], 1.0)

            aq_sb, ak_sb = [], []
            for g in range(4):
                t = p_const.tile([P, 1], F32, name=f"aq{g}")
                nc.sync.dma_start(out=t, in_=aq_d[g * P:(g + 1) * P, 0:1])
                aq_sb.append(t)
                t = p_const.tile([P, 1], F32, name=f"ak{g}")
                nc.sync.dma_start(out=t, in_=ak_d[g * P:(g + 1) * P, 0:1])
                ak_sb.append(t)
            av_sb = p_const.tile([1, HD], F32, name="av_sb")
            nc.sync.dma_start(out=av_sb, in_=av_d[0:1, :])
            vec_sb = {}
            for vn, dram in vecs.items():
                n0 = dram.shape[0]
                if n0 == 1:
                    t = p_const.tile([1, dram.shape[1]], F32, name=vn)
                    nc.sync.dma_start(out=t, in_=dram)
                    vec_sb[vn] = t
                else:
                    tiles = []
                    for k in range(n0 // P):
                        t = p_const.tile([P, 1], F32, name=f"{vn}{k}")
                        nc.sync.dma_start(out=t, in_=dram[k * P:(k + 1) * P, 0:1])
                        tiles.append(t)
                    vec_sb[vn] = tiles

            # persistent activations
            xb = []
            for k in range(KT):
                t = p_x.tile([P, N], BF16, name=f"xb{k}")
                nc.sync.dma_start(out=t, in_=xb_d[k * P:(k + 1) * P, :])
                xb.append(t)
            q_sb = [p_qk.tile([P, N], BF16, name=f"q{g}") for g in range(4)]
            k_sb = [p_qk.tile([P, N], BF16, name=f"k{g}") for g in range(4)]
            vtok = [p_v.tile([P, H * DA], BF16, name=f"v{mt}") for mt in range(KT)]
            # LN1 broadcast tiles (persist through both W1 epilogues)
            rstd1_b = p_bc.tile([P, N], F32, name="rstd1_b")
            mr1n_b = p_bc.tile([P, N], F32, name="mr1n_b")

            def ln_rows(src_tiles, sq_tiles, Nn, pref, rot_pool, rot_tag,
                        rows_pool):
                """Feature-axis LN stats -> (rstd_row, mrn_row) [1, Nn] f32.
                mrn = -mu*rstd."""
                nchunk = Nn // 512
                nk = len(src_tiles)
                s1 = rows_pool.tile([1, Nn], F32, name=f"{pref}_s1", tag="row")
                s2 = rows_pool.tile([1, Nn], F32, name=f"{pref}_s2", tag="row")
                for dst, rhs in ((s1, src_tiles), (s2, sq_tiles)):
                    for cb in range(nchunk):
                        ps = rot_pool.tile([P, 512], F32, name=f"{pref}_sps",
                                           tag=rot_tag)
                        for k in range(nk):
                            nc.tensor.matmul(
                                ps[0:1, :], ones_col[:, 0:1],
                                rhs[k][:, cb * 512:(cb + 1) * 512],
                                start=(k == 0), stop=(k == nk - 1))
                        nc.vector.tensor_copy(
                            out=dst[0:1, cb * 512:(cb + 1) * 512], in_=ps[0:1, :])
                mun = rows_pool.tile([1, Nn], F32, name=f"{pref}_mun", tag="row")
                nc.vector.tensor_scalar_mul(out=mun, in0=s1, scalar1=-1.0 / C)
                ex2 = rows_pool.tile([1, Nn], F32, name=f"{pref}_ex2", tag="row")
                nc.vector.tensor_scalar_mul(out=ex2, in0=s2, scalar1=1.0 / C)
                mu2 = s1  # reuse
                nc.vector.tensor_mul(out=mu2[0:1, :], in0=mun, in1=mun)
                var = s2  # reuse
                nc.vector.tensor_tensor(out=var[0:1, :], in0=ex2, in1=mu2[0:1, :],
                                        op=OP.subtract)
                lnv = ex2  # reuse
                nc.scalar.activation(out=lnv, in_=var[0:1, :], func=AF.Ln,
                                     bias=eps1[0:1, 0:1])
                rstd = rows_pool.tile([1, Nn], F32, name=f"{pref}_rstd",
                                      tag="row")
                nc.scalar.activation(out=rstd, in_=lnv, func=AF.Exp, scale=-0.5)
                mrn = mun  # reuse (mun dead after this)
                nc.vector.tensor_mul(out=mrn[0:1, :], in0=mun, in1=rstd)
                return rstd, mrn

            def bcast(row, dst, Nn, rot_pool, rot_tag, pref):
                for cb in range(Nn // 512):
                    ps = rot_pool.tile([P, 512], F32, name=f"{pref}_bc",
                                       tag=rot_tag)
                    nc.tensor.matmul(ps, ones_row[0:1, :],
                                     row[0:1, cb * 512:(cb + 1) * 512],
                                     start=True, stop=True)
                    nc.vector.tensor_copy(out=dst[:, cb * 512:(cb + 1) * 512],
                                          in_=ps)

            # ================= PHASE 1: stats + QKV =================
            with ExitStack() as es1:
                p_y = es1.enter_context(tc.tile_pool(name="p_y", bufs=1))
                p_w = es1.enter_context(tc.tile_pool(name="p_w", bufs=16))
                p_sq = es1.enter_context(tc.tile_pool(name="p_sq", bufs=3))
                p_rows1 = es1.enter_context(
                    tc.tile_pool(name="p_rows1", bufs=5))
                p_bc2 = es1.enter_context(tc.tile_pool(name="p_bc2", bufs=1))
                ps1 = es1.enter_context(
                    tc.tile_pool(name="ps1", bufs=6, space="PSUM"))
                yb = []
                for k in range(KT):
                    t = p_y.tile([P, N], BF16, name=f"yb{k}")
                    nc.sync.dma_start(out=t, in_=yb_d[k * P:(k + 1) * P, :])
                    yb.append(t)
                wq_sb, wk_sb = [], []
                for wn, dram, lst in (("wq", wq_d, wq_sb), ("wk", wk_d, wk_sb)):
                    for k in range(KT):
                        t = p_w.tile([P, HD], BF16, name=f"{wn}s{k}", tag="w")
                        nc.gpsimd.dma_start(out=t, in_=dram[k * P:(k + 1) * P, :])
                        lst.append(t)

                # squares on ACT (square is in every table set)
                sqx, sqy = [], []
                for nm, src, lst in (("sx", xb, sqx), ("sy", yb, sqy)):
                    for k in range(KT):
                        t = p_sq.tile([P, N], BF16, name=f"{nm}{k}", tag="sq")
                        nc.scalar.activation(out=t, in_=src[k], func=AF.Square)
                        lst.append(t)

                rstd1_r, mr1n_r = ln_rows(xb, sqx, N, "l1", ps1, "r", p_rows1)
                rstd2_r, mr2n_r = ln_rows(yb, sqy, N, "l2", ps1, "r", p_rows1)

                bcast(rstd1_r, rstd1_b, N, ps1, "r", "r1")
                bcast(mr1n_r, mr1n_b, N, ps1, "r", "m1")
                rstd2_b = p_bc2.tile([P, N], F32, name="rstd2_b")
                mr2n_b = p_bc2.tile([P, N], F32, name="mr2n_b")
                bcast(rstd2_r, rstd2_b, N, ps1, "r", "r2")
                bcast(mr2n_r, mr2n_b, N, ps1, "r", "m2")

                # rstd2 transposed to per-token columns (for v correction)
                colp = ps1.tile([P, 512], F32, name="colp", tag="r")
                for mt in range(KT):
                    nc.tensor.matmul(colp[:, mt:mt + 1],
                                     rstd2_r[0:1, mt * P:(mt + 1) * P],
                                     ones1[0:1, 0:1], start=True, stop=True)
                rstd2c = p_bc2.tile([P, KT], F32, name="rstd2c")
                nc.vector.tensor_copy(out=rstd2c, in_=colp[:, 0:KT])

                # q/k projections off raw x/y + per-token correction
                def qk_group(dst, wsb, src, acol, bvec, gg, nb, rbc, mbc):
                    ns = slice(nb * 512, (nb + 1) * 512)
                    ps = ps1.tile([P, 512], F32, name=f"qk{gg}{nb}", tag="r")
                    for k in range(KT):
                        nc.tensor.matmul(ps, wsb[k][:, gg * P:(gg + 1) * P],
                                         src[k][:, ns],
                                         start=(k == 0), stop=(k == KT - 1))
                    t = p_tmp.tile([P, 512], F32, name=f"t{gg}{nb}", tag="t")
                    nc.vector.tensor_mul(out=t, in0=ps, in1=rbc[:, ns])
                    nc.vector.scalar_tensor_tensor(
                        out=dst[gg][:, ns], in0=mbc[:, ns], scalar=acol[gg],
                        in1=t, op0=OP.mult, op1=OP.add)
                    if bvec is not None:
                        nc.vector.tensor_scalar_add(
                            out=dst[gg][:, ns], in0=dst[gg][:, ns],
                            scalar1=bvec[gg])

                bq_t = vec_sb.get("bq")
                bk_t = vec_sb.get("bk")
                for g in range(4):
                    qk_group(k_sb, wk_sb, yb, ak_sb, bk_t, g, 0, rstd2_b, mr2n_b)
                    qk_group(k_sb, wk_sb, yb, ak_sb, bk_t, g, 1, rstd2_b, mr2n_b)
                    qk_group(q_sb, wq_sb, xb, aq_sb, bq_t, g, 0, rstd1_b, mr1n_b)
                for g in range(4):
                    qk_group(q_sb, wq_sb, xb, aq_sb, bq_t, g, 1, rstd1_b, mr1n_b)

                # v (token-major) + per-token correction, packed with ones
                # cols; outer product mr2n (x) av (+ 1 (x) bv) via K=1 matmuls
                wv_sb = []
                for k in range(KT):
                    t = p_w.tile([P, HD], BF16, name=f"wvs{k}", tag="w")
                    nc.gpsimd.dma_start(out=t, in_=wv_d[k * P:(k + 1) * P, :])
                    wv_sb.append(t)
                for mt in range(KT):
                    vp = ps1.tile([P, HD], F32, name=f"vp{mt}", tag="r")
                    for k in range(KT):
                        nc.tensor.matmul(vp, yb[k][:, mt * P:(mt + 1) * P],
                                         wv_sb[k],
                                         start=(k == 0), stop=(k == KT - 1))
                    op_ = ps1.tile([P, HD], F32, name=f"op{mt}", tag="r")
                    nc.tensor.matmul(op_, mr2n_r[0:1, mt * P:(mt + 1) * P],
                                     av_sb[0:1, :], start=True,
                                     stop=not f_bkv)
                    if f_bkv:
                        nc.tensor.matmul(op_, ones_row[0:1, :],
                                         vec_sb["bv"][0:1, :], start=False,
                                         stop=True)
                    oc = p_tmp.tile([P, HD], F32, name=f"oc{mt}", tag="t")
                    nc.vector.tensor_copy(out=oc, in_=op_)
                    vt = vtok[mt]
                    nc.vector.memset(vt, 1.0)
                    vt3 = vt.rearrange("p (h w) -> p h w", w=DA)
                    nc.vector.scalar_tensor_tensor(
                        out=vt3[:, :, 0:D],
                        in0=vp.rearrange("p (h w) -> p h w", w=D),
                        scalar=rstd2c[:, mt:mt + 1],
                        in1=oc.rearrange("p (h w) -> p h w", w=D),
                        op0=OP.mult, op1=OP.add)

            # ============ PHASE 2: attention + pipelined FFN ============
            out1 = {}
            ln3t = {}
            parked = []
            with ExitStack() as es2:
                p_sc = es2.enter_context(
                    tc.tile_pool(name="p_sc", bufs=2, space="PSUM"))
                p_aug = es2.enter_context(
                    tc.tile_pool(name="p_aug", bufs=1, space="PSUM"))
                ps2 = es2.enter_context(
                    tc.tile_pool(name="ps2", bufs=2, space="PSUM"))
                p_e = es2.enter_context(tc.tile_pool(name="p_e", bufs=3))
                p_oT = es2.enter_context(tc.tile_pool(name="p_oT", bufs=8))
                p_zz = es2.enter_context(tc.tile_pool(name="p_zz", bufs=2))
                p_sq3 = es2.enter_context(tc.tile_pool(name="p_sq3", bufs=8))
                p_bc3 = es2.enter_context(tc.tile_pool(name="p_bc3", bufs=1))
                p_w2 = es2.enter_context(tc.tile_pool(name="p_w2", bufs=10))
                p_rows3 = es2.enter_context(
                    tc.tile_pool(name="p_rows3", bufs=5))
                p_w1p = es2.enter_context(tc.tile_pool(name="p_w1p", bufs=1))
                w1p_sb = []
                for i in range(8):
                    t = p_w1p.tile([P, C], BF16, name=f"w1p{i}")
                    nc.gpsimd.dma_start(out=t, in_=w1p_d[i * P:(i + 1) * P, :])
                    w1p_sb.append(t)
                aug_cur = {}
                oT = {}

                def attn_group(nb, g, mt):
                    ns = slice(nb * 512, (nb + 1) * 512)
                    if mt == 0:
                        aug_cur[0] = p_aug.tile([P, 512], F32,
                                                name=f"a01_{nb}{g}", tag="a01")
                        aug_cur[1] = p_aug.tile([P, 512], F32,
                                                name=f"a23_{nb}{g}", tag="a23")
                    for pr in range(2):
                        s_ps = p_sc.tile([P, 1024], F32,
                                         name=f"s{nb}{g}{mt}{pr}", tag="sc")
                        for jj in range(2):
                            j = 2 * pr + jj
                            rs = slice(32 * j, 32 * (j + 1))
                            nc.tensor.matmul(
                                s_ps[:, jj * 512:(jj + 1) * 512],
                                k_sb[g][rs, mt * P:(mt + 1) * P],
                                q_sb[g][rs, ns], start=True, stop=True,
                                tile_position=(32 * j, 0))
                        et = p_e.tile([P, 1024], BF16,
                                      name=f"e{nb}{g}{mt}{pr}", tag="e")
                        nc.scalar.activation(out=et, in_=s_ps, func=AF.Exp)
                        for jj in range(2):
                            h = 4 * g + 2 * pr + jj
                            nc.tensor.matmul(
                                aug_cur[pr][64 * jj:64 * (jj + 1), :],
                                vtok[mt][:, h * DA:(h + 1) * DA],
                                et[:, jj * 512:(jj + 1) * 512],
                                start=(mt == 0), stop=(mt == KT - 1),
                                tile_position=(0, 64 * jj))

                def attn_drain(nb, g):
                    for pr in range(2):
                        zr = p_zz.tile([P, 512], F32, name=f"zr{nb}{g}{pr}",
                                       tag="zr", bufs=1)
                        nc.vector.memset(zr, 0.0)
                        nc.vector.tensor_copy(out=zr[32:33, :],
                                              in_=aug_cur[pr][32:33, :])
                        nc.vector.tensor_copy(out=zr[96:97, :],
                                              in_=aug_cur[pr][96:97, :])
                        zb = ps2.tile([P, 512], F32, name=f"zb{nb}{g}{pr}",
                                      tag="r")
                        nc.tensor.matmul(zb, e2, zr, start=True, stop=True)
                        zi = p_zz.tile([P, 512], F32, name=f"zi{nb}{g}{pr}",
                                       tag="zi")
                        nc.vector.reciprocal_approx_fast(out=zi, in_=zb)
                        o = p_oT.tile([P, 512], BF16, name=f"o{nb}{g}{pr}",
                                      tag="oT")
                        nc.vector.tensor_mul(out=o, in0=aug_cur[pr], in1=zi)
                        oT[(nb, 2 * g + pr)] = o

                def w1_ct(nb, ct):
                    ns = slice(nb * 512, (nb + 1) * 512)
                    u_ps = ps2.tile([P, 512], F32, name=f"u{nb}{ct}", tag="r")
                    for i in range(8):
                        nc.tensor.matmul(u_ps, w1p_sb[i][:, ct * P:(ct + 1) * P],
                                         oT[(nb, i)],
                                         start=(i == 0), stop=(i == 7))
                    t = p_tmp.tile([P, 512], F32, name=f"x{nb}{ct}", tag="t")
                    nc.vector.tensor_mul(out=t, in0=xb[ct][:, ns],
                                         in1=rstd1_b[:, ns])
                    t2 = p_tmp.tile([P, 512], F32, name=f"x2{nb}{ct}", tag="t")
                    nc.vector.tensor_tensor(out=t2, in0=t, in1=mr1n_b[:, ns],
                                            op=OP.add)
                    if f_a1:
                        nc.vector.tensor_scalar(
                            out=t2, in0=t2, scalar1=vec_sb["g1"][ct],
                            scalar2=vec_sb["be1"][ct], op0=OP.mult, op1=OP.add)
                    o1 = p_out1.tile([P, 512], BF16, name=f"o1_{nb}{ct}",
                                     tag="o1")
                    b1s = vec_sb["b1"][ct] if f_b1 else 0.0
                    nc.vector.scalar_tensor_tensor(
                        out=o1, in0=u_ps, scalar=b1s, in1=t2,
                        op0=OP.add, op1=OP.add)
                    out1[(nb, ct)] = o1
                    sq3 = p_sq3.tile([P, 512], BF16, name=f"sq3_{nb}{ct}",
                                     tag="s3")
                    nc.vector.tensor_mul(out=sq3, in0=o1, in1=o1)
                    return sq3

                def ln3_block(nb, sq3s):
                    srcs = [out1[(nb, ct)] for ct in range(KT)]
                    rstd3_r, mr3n_r = ln_rows(srcs, sq3s, 512, f"l3{nb}",
                                              ps2, "r", p_rows3)
                    rstd3_b = p_bc3.tile([P, 512], F32, name=f"r3b{nb}",
                                         tag="r3")
                    mr3n_b = p_bc3.tile([P, 512], F32, name=f"m3b{nb}",
                                        tag="m3")
                    bcast(rstd3_r, rstd3_b, 512, ps2, "r", f"b3{nb}")
                    bcast(mr3n_r, mr3n_b, 512, ps2, "r", f"c3{nb}")
                    for ct in range(KT):
                        t = p_tmp.tile([P, 512], F32, name=f"l3t{nb}{ct}",
                                       tag="t")
                        nc.vector.tensor_mul(out=t, in0=out1[(nb, ct)],
                                             in1=rstd3_b)
                        l3 = p_ln3.tile([P, 512], BF16, name=f"l3_{nb}{ct}",
                                        tag="l3")
                        nc.vector.tensor_tensor(out=l3, in0=t, in1=mr3n_b,
                                                op=OP.add)
                        ln3t[(nb, ct)] = l3

                w2t_cur = []

                def w2_slot(nb, ft, park):
                    fq, fi = ft // 4, ft % 4
                    if fi == 0:
                        w2t_cur.clear()
                        for ct in range(KT):
                            t = p_w2.tile([P, 512], BF16,
                                          name=f"w2_{nb}{fq}{ct}", tag="w2")
                            nc.sync.dma_start(
                                out=t, in_=w2_d[ct * P:(ct + 1) * P,
                                               fq * 512:(fq + 1) * 512])
                            w2t_cur.append(t)
                    h_ps = ps2.tile([P, 512], F32, name=f"h{nb}{ft}", tag="r")
                    for ct in range(KT):
                        nc.tensor.matmul(h_ps,
                                         w2t_cur[ct][:, fi * P:(fi + 1) * P],
                                         ln3t[(nb, ct)],
                                         start=(ct == 0), stop=(ct == KT - 1))
                    hp = p_park.tile([P, 512], BF16, name=f"hp{nb}{ft}",
                                     tag="pk")
                    if park:
                        nc.vector.tensor_copy(out=hp, in_=h_ps)
                    else:
                        b2s = vec_sb["b2"][ft] if f_b2 else 0.0
                        nc.scalar.activation(out=hp, in_=h_ps, func=AF.Gelu,
                                             bias=b2s)
                    return hp

                # ---- attention block A ----
                for g in range(4):
                    for mt in range(KT):
                        attn_group(0, g, mt)
                    attn_drain(0, g)

                # ---- W1(A) + LN3(A) woven with early attention(B) ----
                groups_b = [(g, mt) for g in range(4) for mt in range(KT)]
                bi = 0

                def feed_b(n_groups):
                    nonlocal bi
                    for _ in range(n_groups):
                        if bi < len(groups_b):
                            g, mt = groups_b[bi]
                            attn_group(1, g, mt)
                            if mt == KT - 1:
                                attn_drain(1, g)
                            bi += 1

                sq3a = []
                for ct in range(KT):
                    sq3a.append(w1_ct(0, ct))
                    feed_b(1)
                ln3_block(0, sq3a)
                feed_b(2)

                # ---- W2(A) (park, no gelu) interleaved with attention(B) ----
                h1a = []
                for ft in range(32):
                    h1a.append(w2_slot(0, ft, park=True))
                    feed_b(1)
                feed_b(len(groups_b))  # flush any remainder

                # ---- W1(B) + LN3(B) (exp set still loaded) ----
                sq3b = [w1_ct(1, ct) for ct in range(KT)]
                ln3_block(1, sq3b)

            # ---- gelu(A) + W3(A) ----
            with ExitStack() as es3:
                p_h2a = es3.enter_context(
                    tc.tile_pool(name="p_h2a", bufs=1, space="PSUM"))
                p_hg = es3.enter_context(tc.tile_pool(name="p_hg", bufs=6))
                p_w3 = es3.enter_context(tc.tile_pool(name="p_w3", bufs=3))
                p_fin = es3.enter_context(tc.tile_pool(name="p_fin", bufs=4))
                def w3_block(nb, hg_of_ft, h2_pool):
                    ns = slice(nb * 512, (nb + 1) * 512)
                    h2 = [h2_pool.tile([P, 512], F32, name=f"h2_{nb}{ct}",
                                       tag=f"h2{ct}") for ct in range(KT)]
                    for ft in range(32):
                        w3t = p_w3.tile([P, C], BF16, name=f"w3_{nb}{ft}",
                                        tag="w3")
                        nc.sync.dma_start(out=w3t,
                                          in_=w3_d[ft * P:(ft + 1) * P, :])
                        hg = hg_of_ft(ft)
                        for ct in range(KT):
                            nc.tensor.matmul(h2[ct],
                                             w3t[:, ct * P:(ct + 1) * P], hg,
                                             start=(ft == 0), stop=(ft == 31))
                    for ct in range(KT):
                        fin = p_fin.tile([P, 512], F32, name=f"f{nb}{ct}",
                                         tag="fin")
                        b3s = vec_sb["b3"][ct] if f_b3 else 0.0
                        nc.vector.scalar_tensor_tensor(
                            out=fin, in0=h2[ct], scalar=b3s,
                            in1=out1[(nb, ct)], op0=OP.add, op1=OP.add)
                        nc.sync.dma_start(out=OT[ct * P:(ct + 1) * P, ns],
                                          in_=fin)

                def gelu_a(ft):
                    hg = p_hg.tile([P, 512], BF16, name=f"hg{ft}", tag="hg")
                    b2s = vec_sb["b2"][ft] if f_b2 else 0.0
                    nc.scalar.activation(out=hg, in_=h1a[ft], func=AF.Gelu,
                                         bias=b2s)
                    return hg

                w3_block(0, gelu_a, p_h2a)

            # ---- W2(B) (gelu inline) + W3(B) ----
            with tc.tile_pool(name="ps4", bufs=2, space="PSUM") as ps4:
                hgb = []
                with tc.tile_pool(name="p_w2b", bufs=12) as p_w2b:
                    w2t_b = []

                    def w2_slot_b(ft):
                        fq, fi = ft // 4, ft % 4
                        if fi == 0:
                            w2t_b.clear()
                            for ct in range(KT):
                                t = p_w2b.tile([P, 512], BF16,
                                               name=f"w2b_{fq}{ct}", tag="w2b")
                                nc.sync.dma_start(
                                    out=t, in_=w2_d[ct * P:(ct + 1) * P,
                                                   fq * 512:(fq + 1) * 512])
                                w2t_b.append(t)
                        h_ps = ps4.tile([P, 512], F32, name=f"hb{ft}", tag="r")
                        for ct in range(KT):
                            nc.tensor.matmul(
                                h_ps, w2t_b[ct][:, fi * P:(fi + 1) * P],
                                ln3t[(1, ct)],
                                start=(ct == 0), stop=(ct == KT - 1))
                        hp = p_park.tile([P, 512], BF16, name=f"hpb{ft}",
                                         tag="pk")
                        b2s = vec_sb["b2"][ft] if f_b2 else 0.0
                        nc.scalar.activation(out=hp, in_=h_ps, func=AF.Gelu,
                                             bias=b2s)
                        return hp

                    for ft in range(32):
                        hgb.append(w2_slot_b(ft))

            with ExitStack() as es4:
                p_h2b = es4.enter_context(
                    tc.tile_pool(name="p_h2b", bufs=1, space="PSUM"))
                p_w3b = es4.enter_context(tc.tile_pool(name="p_w3b", bufs=3))
                p_finb = es4.enter_context(tc.tile_pool(name="p_finb", bufs=4))
                h2 = [p_h2b.tile([P, 512], F32, name=f"h2b{ct}",
                                 tag=f"h2b{ct}") for ct in range(KT)]
                ns = slice(512, 1024)
                for ft in range(32):
                    w3t = p_w3b.tile([P, C], BF16, name=f"w3b_{ft}", tag="w3b")
                    nc.sync.dma_start(out=w3t, in_=w3_d[ft * P:(ft + 1) * P, :])
                    for ct in range(KT):
                        nc.tensor.matmul(h2[ct], w3t[:, ct * P:(ct + 1) * P],
                                         hgb[ft],
                                         start=(ft == 0), stop=(ft == 31))
                for ct in range(KT):
                    fin = p_finb.tile([P, 512], F32, name=f"fb{ct}", tag="fin")
                    b3s = vec_sb["b3"][ct] if f_b3 else 0.0
                    nc.vector.scalar_tensor_tensor(
                        out=fin, in0=h2[ct], scalar=b3s, in1=out1[(1, ct)],
                        op0=OP.add, op1=OP.add)
                    nc.sync.dma_start(out=OT[ct * P:(ct + 1) * P, ns], in_=fin)

    nc.finalize()
    return nc


def _nontrivial(v, val):
    return not np.allclose(np.asarray(v), val, rtol=0.0, atol=0.0)


def kernel(x, y, Wq, Wk, Wv, W1, b1, g1, be1, g2, be2, g3, be3, W2, b2, W3, b3):
    x = np.asarray(x, np.float32)
    y = np.asarray(y, np.float32)
    bf = ml_dtypes.bfloat16
    g1 = np.asarray(g1, np.float32)
    be1 = np.asarray(be1, np.float32)
    g2 = np.asarray(g2, np.float32)
    be2 = np.asarray(be2, np.float32)
    g3 = np.asarray(g3, np.float32)
    be3 = np.asarray(be3, np.float32)

    f_a1 = _nontrivial(g1, 1.0) or _nontrivial(be1, 0.0)
    f_bq = _nontrivial(be1, 0.0)
    f_bkv = _nontrivial(be2, 0.0)
    f_b1 = _nontrivial(b1, 0.0)
    f_b2 = _nontrivial(b2, 0.0) or _nontrivial(be3, 0.0)
    f_b3 = _nontrivial(b3, 0.0)
    flags = (f_a1, f_bq, f_bkv, f_b1, f_b2, f_b3)

    if flags not in _BUILD_CACHE:
        _BUILD_CACHE[flags] = _build(flags)
    nc = _BUILD_CACHE[flags]

    # host-side weight prep (g folded in; attention scale folded into Wq)
    wq_e = (np.transpose(np.asarray(Wq, np.float32), (1, 0, 2)).reshape(C, HD)
            * g1[:, None] * (D ** -0.5))
    wk_e = (np.transpose(np.asarray(Wk, np.float32), (1, 0, 2)).reshape(C, HD)
            * g2[:, None])
    wv_e = (np.transpose(np.asarray(Wv, np.float32), (1, 0, 2)).reshape(C, HD)
            * g2[:, None])
    aq = wq_e.sum(axis=0).reshape(HD, 1)
    ak = wk_e.sum(axis=0).reshape(HD, 1)
    av = wv_e.sum(axis=0).reshape(1, HD).astype(np.float32)

    W1f = np.asarray(W1, np.float32)
    w1p = np.zeros((2 * HD, C), np.float32)
    for g in range(4):
        for pr in range(2):
            i = 2 * g + pr
            h0 = 4 * g + 2 * pr
            h1 = h0 + 1
            w1p[i * 128:i * 128 + 32] = W1f[h0 * 32:(h0 + 1) * 32]
            w1p[i * 128 + 64:i * 128 + 96] = W1f[h1 * 32:(h1 + 1) * 32]

    w2_e = np.asarray(W2, np.float32) * g3[:, None]
    b2e = np.asarray(b2, np.float32) + be3 @ np.asarray(W2, np.float32)

    m0 = {
        "wq": wq_e.astype(bf), "wk": wk_e.astype(bf), "wv": wv_e.astype(bf),
        "aq": np.ascontiguousarray(aq), "ak": np.ascontiguousarray(ak),
        "av": np.ascontiguousarray(av),
        "w1p": w1p.astype(bf), "w2": w2_e.astype(bf),
        "w3": np.asarray(W3, np.float32).astype(bf),
    }
    if f_bq:
        m0["bq"] = (be1 @ wq_e).reshape(HD, 1).astype(np.float32)
    if f_bkv:
        m0["bk"] = (be2 @ wk_e).reshape(HD, 1).astype(np.float32)
        m0["bv"] = (be2 @ wv_e).reshape(1, HD).astype(np.float32)
    if f_a1:
        m0["g1"] = g1.reshape(C, 1)
        m0["be1"] = be1.reshape(C, 1)
    if f_b1:
        m0["b1"] = np.asarray(b1, np.float32).reshape(C, 1)
    if f_b2:
        m0["b2"] = b2e.reshape(F, 1).astype(np.float32)
    if f_b3:
        m0["b3"] = np.asarray(b3, np.float32).reshape(C, 1)

    in_maps = []
    for b in range(NCORES):
        m = dict(m0)
        m["xb"] = np.ascontiguousarray(x[b].T).astype(bf)
        m["yb"] = np.ascontiguousarray(y[b].T).astype(bf)
        in_maps.append(m)

    global _LAST_IN_MAPS
    _LAST_IN_MAPS = in_maps
    res = bass_utils.run_bass_kernel_spmd(nc, in_maps, core_ids=list(range(NCORES)))
    out = np.stack([np.ascontiguousarray(r["OT"].T) for r in res.results])
    return out.astype(np.float32)


# revision 58
# speedup vs baseline: 1.0799x; 1.0799x over previous
"""Trainium2 Bass kernel for the cross-attention transformer block.

Strategy (8 NeuronCores, data-parallel over batch B=8, one batch item per core):

  - LN1/LN2 are FOLDED INTO the QKV projections: q = rstd*(Wq'^T x) -
    (mu*rstd)*colsum(Wq') (+ beta terms), with gamma folded into the weights
    host-side.  The QKV matmuls therefore consume the raw bf16 x/y straight
    from DMA, and the LN stats (ones-vector matmuls) interleave with them so
    the PE array stays dense and warm from the start.
  - rstd is computed as exp(-0.5*ln(var+eps)) so that ONE activation table
    set (natural_log_exp_and_others: exp/ln/square) covers layernorms AND the
    attention softmax; the only table switch in the whole kernel is to gelu.
  - Attention: scores^T = K^T(d,m)Q(d,n) with row-packed (tile_position) pairs
    of heads; softmax skips the max-subtraction (|s|<6 for these shapes); exp
    runs on ACT straight out of PSUM in [128,1024] chunks, double-buffered.
    V carries an appended ones column per head, and the AV matmuls are
    column-packed 2 heads per 128-wide PSUM tile, so each e-tile feeds two
    concurrent col-tiled matmuls.  W1 is host-permuted (with zero rows) to
    match the packed o layout.
  - The token dim is split into two 512-column blocks A/B and software-
    pipelined: FFN(A) (PE-heavy) runs WHILE attention(B)'s exp stream
    (ACT-heavy) executes; gelu(A) is batched after the last exp(B) to avoid
    activation-table thrash.
  - FFN weights (16 MB bf16) are streamed from HBM once per block.
"""

import sys

for _p in ("/opt/trn_rl_repo", "/root/.axon_site/_ro/trn_rl_repo"):
    if _p not in sys.path:
        sys.path.append(_p)

from contextlib import ExitStack

import numpy as np
import ml_dtypes

import concourse.bacc as bacc
import concourse.mybir as mybir
from concourse.tile import TileContext
from concourse import bass_utils
from concourse import hw_specs

F32 = mybir.dt.float32
BF16 = mybir.dt.bfloat16
AF = mybir.ActivationFunctionType
OP = mybir.AluOpType

P = 128
B, N, C, H, D, W = 8, 1024, 1024, 16, 32, 4
HD = H * D            # 512
DA = 2 * D            # 64: V + ones column (padded)
F = W * C             # 4096
KT = C // P           # 8 feature k-tiles
EPS = 1e-5
NCORES = 8

_BUILD_CACHE = {}
_LAST_IN_MAPS = None


def _patch_act_tables(arch):
    """Steer Ln and Exp to the combined natural_log_exp_and_others table set
    (set selection is first-match over act_info.json order, which would
    otherwise pick disjoint sets and pay a table reload per Ln->Exp pair).
    The chosen set genuinely contains exp/ln/square in hardware."""
    tabs = hw_specs.get_activation_tables(arch)
    if "natural_log_exp_and_others" in tabs:
        tabs.get("exp_and_others", set()).discard(AF.Exp)
        tabs.get("natural_log", set()).discard(AF.Ln)


def _build(flags):
    """flags = (f_a1, f_bq, f_bkv, f_b1, f_b2, f_b3)"""
    f_a1, f_bq, f_bkv, f_b1, f_b2, f_b3 = flags
    nc = bacc.Bacc("TRN2", target_bir_lowering=False)
    _patch_act_tables(nc.m.arch)

    xb_d = nc.dram_tensor("xb", [C, N], BF16, kind="ExternalInput")
    yb_d = nc.dram_tensor("yb", [C, N], BF16, kind="ExternalInput")
    wq_d = nc.dram_tensor("wq", [C, HD], BF16, kind="ExternalInput")
    wk_d = nc.dram_tensor("wk", [C, HD], BF16, kind="ExternalInput")
    wv_d = nc.dram_tensor("wv", [C, HD], BF16, kind="ExternalInput")
    aq_d = nc.dram_tensor("aq", [HD, 1], F32, kind="ExternalInput")
    ak_d = nc.dram_tensor("ak", [HD, 1], F32, kind="ExternalInput")
    av_d = nc.dram_tensor("av", [1, HD], F32, kind="ExternalInput")
    w1p_d = nc.dram_tensor("w1p", [2 * HD, C], BF16, kind="ExternalInput")
    w2_d = nc.dram_tensor("w2", [C, F], BF16, kind="ExternalInput")
    w3_d = nc.dram_tensor("w3", [F, C], BF16, kind="ExternalInput")
    vecs = {}
    if f_bq:
        vecs["bq"] = nc.dram_tensor("bq", [HD, 1], F32, kind="ExternalInput")
    if f_bkv:
        vecs["bk"] = nc.dram_tensor("bk", [HD, 1], F32, kind="ExternalInput")
        vecs["bv"] = nc.dram_tensor("bv", [1, HD], F32, kind="ExternalInput")
    if f_a1:
        vecs["g1"] = nc.dram_tensor("g1", [C, 1], F32, kind="ExternalInput")
        vecs["be1"] = nc.dram_tensor("be1", [C, 1], F32, kind="ExternalInput")
    if f_b1:
        vecs["b1"] = nc.dram_tensor("b1", [C, 1], F32, kind="ExternalInput")
    if f_b2:
        vecs["b2"] = nc.dram_tensor("b2", [F, 1], F32, kind="ExternalInput")
    if f_b3:
        vecs["b3"] = nc.dram_tensor("b3", [C, 1], F32, kind="ExternalInput")
    OT = nc.dram_tensor("OT", [C, N], F32, kind="ExternalOutput")

    with TileContext(nc) as tc:
        with ExitStack() as top_es:
            def pool(name, bufs, space=None):
                kw = {"space": space} if space else {}
                return top_es.enter_context(
                    tc.tile_pool(name=name, bufs=bufs, **kw))

            p_const = pool("p_const", 1)
            p_x = pool("p_x", 1)
            p_qk = pool("p_qk", 1)
            p_v = pool("p_v", 1)
            p_bc = pool("p_bc", 1)
            p_out1 = pool("p_out1", 16)
            p_park = pool("p_park", 32)
            p_tmp = pool("p_tmp", 4)
            p_ln3 = pool("p_ln3", 8)
            # ---------------- constants ----------------
            ones_col = p_const.tile([P, 1], BF16, name="ones_col")
            nc.vector.memset(ones_col, 1.0)
            ones_row = p_const.tile([1, P], F32, name="ones_row")
            nc.vector.memset(ones_row, 1.0)
            ones1 = p_const.tile([1, 1], F32, name="ones1")
            nc.vector.memset(ones1, 1.0)
            eps1 = p_const.tile([1, 1], F32, name="eps1")
            nc.vector.memset(eps1, EPS)
            # E2[k,p] = 1 iff (k==32 and p<64) or (k==96 and p>=64):
            # broadcasts the per-head Z rows of a packed aug tile
            e2 = p_const.tile([P, P], F32, name="e2")
            nc.vector.memset(e2, 0.0)
            nc.vector.memset(e2[32:33, 0:64], 1.0)
            nc.vector.memset(e2[96:97, 64:128], 1.0)

            aq_sb, ak_sb = [], []
            for g in range(4):
                t = p_const.tile([P, 1], F32, name=f"aq{g}")
                nc.sync.dma_start(out=t, in_=aq_d[g * P:(g + 1) * P, 0:1])
                aq_sb.append(t)
                t = p_const.tile([P, 1], F32, name=f"ak{g}")
                nc.sync.dma_start(out=t, in_=ak_d[g * P:(g + 1) * P, 0:1])
                ak_sb.append(t)
            av_sb = p_const.tile([1, HD], F32, name="av_sb")
            nc.sync.dma_start(out=av_sb, in_=av_d[0:1, :])
            vec_sb = {}
            for vn, dram in vecs.items():
                n0 = dram.shape[0]
                if n0 == 1:
                    t = p_const.tile([1, dram.shape[1]], F32, name=vn)
                    nc.sync.dma_start(out=t, in_=dram)
                    vec_sb[vn] = t
                else:
                    tiles = []
                    for k in range(n0 // P):
                        t = p_const.tile([P, 1], F32, name=f"{vn}{k}")
                        nc.sync.dma_start(out=t, in_=dram[k * P:(k + 1) * P, 0:1])
                        tiles.append(t)
                    vec_sb[vn] = tiles

            # persistent activations
            xb = []
            for k in range(KT):
                t = p_x.tile([P, N], BF16, name=f"xb{k}")
                nc.sync.dma_start(out=t, in_=xb_d[k * P:(k + 1) * P, :])
                xb.append(t)
            q_sb = [p_qk.tile([P, N], BF16, name=f"q{g}") for g in range(4)]
            k_sb = [p_qk.tile([P, N], BF16, name=f"k{g}") for g in range(4)]
            vtok = [p_v.tile([P, H * DA], BF16, name=f"v{mt}") for mt in range(KT)]
            # LN1 broadcast tiles (persist through both W1 epilogues)
            rstd1_b = p_bc.tile([P, N], F32, name="rstd1_b")
            mr1n_b = p_bc.tile([P, N], F32, name="mr1n_b")

            def ln_rows(src_tiles, sq_tiles, Nn, pref, rot_pool, rot_tag,
                        rows_pool):
                """Feature-axis LN stats -> (rstd_row, mrn_row) [1, Nn] f32.
                mrn = -mu*rstd."""
                nchunk = Nn // 512
                nk = len(src_tiles)
                s1 = rows_pool.tile([1, Nn], F32, name=f"{pref}_s1", tag="tmp",
                                    bufs=3)
                s2 = rows_pool.tile([1, Nn], F32, name=f"{pref}_s2", tag="tmp",
                                    bufs=3)
                for dst, rhs in ((s1, src_tiles), (s2, sq_tiles)):
                    for cb in range(nchunk):
                        ps = rot_pool.tile([P, 512], F32, name=f"{pref}_sps",
                                           tag=rot_tag)
                        for k in range(nk):
                            nc.tensor.matmul(
                                ps[0:1, :], ones_col[:, 0:1],
                                rhs[k][:, cb * 512:(cb + 1) * 512],
                                start=(k == 0), stop=(k == nk - 1))
                        nc.vector.tensor_copy(
                            out=dst[0:1, cb * 512:(cb + 1) * 512], in_=ps[0:1, :])
                mun = rows_pool.tile([1, Nn], F32, name=f"{pref}_mun",
                                     tag="keep", bufs=2)
                nc.vector.tensor_scalar_mul(out=mun, in0=s1, scalar1=-1.0 / C)
                ex2 = rows_pool.tile([1, Nn], F32, name=f"{pref}_ex2", tag="tmp",
                                     bufs=3)
                nc.vector.tensor_scalar_mul(out=ex2, in0=s2, scalar1=1.0 / C)
                mu2 = s1  # reuse
                nc.vector.tensor_mul(out=mu2[0:1, :], in0=mun, in1=mun)
                var = s2  # reuse
                nc.vector.tensor_tensor(out=var[0:1, :], in0=ex2, in1=mu2[0:1, :],
                                        op=OP.subtract)
                lnv = ex2  # reuse
                nc.scalar.activation(out=lnv, in_=var[0:1, :], func=AF.Ln,
                                     bias=eps1[0:1, 0:1])
                rstd = rows_pool.tile([1, Nn], F32, name=f"{pref}_rstd",
                                      tag="krstd", bufs=2)
                nc.scalar.activation(out=rstd, in_=lnv, func=AF.Exp, scale=-0.5)
                mrn = mun  # reuse (mun dead after this)
                nc.vector.tensor_mul(out=mrn[0:1, :], in0=mun, in1=rstd)
                return rstd, mrn

            def bcast(row, dst, Nn, rot_pool, rot_tag, pref):
                for cb in range(Nn // 512):
                    ps = rot_pool.tile([P, 512], F32, name=f"{pref}_bc",
                                       tag=rot_tag)
                    nc.tensor.matmul(ps, ones_row[0:1, :],
                                     row[0:1, cb * 512:(cb + 1) * 512],
                                     start=True, stop=True)
                    nc.vector.tensor_copy(out=dst[:, cb * 512:(cb + 1) * 512],
                                          in_=ps)

            # ================= PHASE 1: stats + QKV =================
            with ExitStack() as es1:
                p_y = es1.enter_context(tc.tile_pool(name="p_y", bufs=1))
                p_w = es1.enter_context(tc.tile_pool(name="p_w", bufs=16))
                p_sq = es1.enter_context(tc.tile_pool(name="p_sq", bufs=3))
                p_rows1 = es1.enter_context(
                    tc.tile_pool(name="p_rows1", bufs=5))
                p_bc2 = es1.enter_context(tc.tile_pool(name="p_bc2", bufs=1))
                ps1 = es1.enter_context(
                    tc.tile_pool(name="ps1", bufs=6, space="PSUM"))
                yb = []
                for k in range(KT):
                    t = p_y.tile([P, N], BF16, name=f"yb{k}")
                    nc.gpsimd.dma_start(out=t, in_=yb_d[k * P:(k + 1) * P, :])
                    yb.append(t)
                wq_sb, wk_sb = [], []
                for wn, dram, lst in (("wq", wq_d, wq_sb), ("wk", wk_d, wk_sb)):
                    for k in range(KT):
                        t = p_w.tile([P, HD], BF16, name=f"{wn}s{k}", tag="w")
                        nc.gpsimd.dma_start(out=t, in_=dram[k * P:(k + 1) * P, :])
                        lst.append(t)

                # squares on ACT (square is in every table set); y first —
                # k/v corrections are needed before q(B)'s
                sqx, sqy = [], []
                for nm, src, lst in (("sy", yb, sqy), ("sx", xb, sqx)):
                    for k in range(KT):
                        t = p_sq.tile([P, N], BF16, name=f"{nm}{k}", tag="sq")
                        nc.scalar.activation(out=t, in_=src[k], func=AF.Square)
                        lst.append(t)

                rstd2_r, mr2n_r = ln_rows(yb, sqy, N, "l2", ps1, "r", p_rows1)
                rstd1_r, mr1n_r = ln_rows(xb, sqx, N, "l1", ps1, "r", p_rows1)

                rstd2_b = p_bc2.tile([P, N], F32, name="rstd2_b")
                mr2n_b = p_bc2.tile([P, N], F32, name="mr2n_b")
                bcast(rstd2_r, rstd2_b, N, ps1, "r", "r2")
                bcast(mr2n_r, mr2n_b, N, ps1, "r", "m2")
                bcast(rstd1_r, rstd1_b, N, ps1, "r", "r1")
                bcast(mr1n_r, mr1n_b, N, ps1, "r", "m1")

                # rstd2 transposed to per-token columns (for v correction)
                colp = ps1.tile([P, 512], F32, name="colp", tag="r")
                for mt in range(KT):
                    nc.tensor.matmul(colp[:, mt:mt + 1],
                                     rstd2_r[0:1, mt * P:(mt + 1) * P],
                                     ones1[0:1, 0:1], start=True, stop=True)
                rstd2c = p_bc2.tile([P, KT], F32, name="rstd2c")
                nc.vector.tensor_copy(out=rstd2c, in_=colp[:, 0:KT])

                # q/k projections off raw x/y + per-token correction
                def qk_group(dst, wsb, src, acol, bvec, gg, nb, rbc, mbc):
                    ns = slice(nb * 512, (nb + 1) * 512)
                    ps = ps1.tile([P, 512], F32, name=f"qk{gg}{nb}", tag="r")
                    for k in range(KT):
                        nc.tensor.matmul(ps, wsb[k][:, gg * P:(gg + 1) * P],
                                         src[k][:, ns],
                                         start=(k == 0), stop=(k == KT - 1))
                    t = p_tmp.tile([P, 512], F32, name=f"t{gg}{nb}", tag="t")
                    nc.vector.tensor_mul(out=t, in0=ps, in1=rbc[:, ns])
                    nc.vector.scalar_tensor_tensor(
                        out=dst[gg][:, ns], in0=mbc[:, ns], scalar=acol[gg],
                        in1=t, op0=OP.mult, op1=OP.add)
                    if bvec is not None:
                        nc.vector.tensor_scalar_add(
                            out=dst[gg][:, ns], in0=dst[gg][:, ns],
                            scalar1=bvec[gg])

                bq_t = vec_sb.get("bq")
                bk_t = vec_sb.get("bk")
                for g in range(4):
                    qk_group(k_sb, wk_sb, yb, ak_sb, bk_t, g, 0, rstd2_b, mr2n_b)
                    qk_group(k_sb, wk_sb, yb, ak_sb, bk_t, g, 1, rstd2_b, mr2n_b)
                    qk_group(q_sb, wq_sb, xb, aq_sb, bq_t, g, 0, rstd1_b, mr1n_b)

                # v (token-major) + per-token correction, packed with ones
                # cols; outer product mr2n (x) av (+ 1 (x) bv) via K=1 matmuls
                wv_sb = []
                for k in range(KT):
                    t = p_w.tile([P, HD], BF16, name=f"wvs{k}", tag="wv",
                                 bufs=8)
                    nc.gpsimd.dma_start(out=t, in_=wv_d[k * P:(k + 1) * P, :])
                    wv_sb.append(t)
                for mt in range(KT):
                    vp = ps1.tile([P, HD], F32, name=f"vp{mt}", tag="r")
                    for k in range(KT):
                        nc.tensor.matmul(vp, yb[k][:, mt * P:(mt + 1) * P],
                                         wv_sb[k],
                                         start=(k == 0), stop=(k == KT - 1))
                    op_ = ps1.tile([P, HD], F32, name=f"op{mt}", tag="r")
                    nc.tensor.matmul(op_, mr2n_r[0:1, mt * P:(mt + 1) * P],
                                     av_sb[0:1, :], start=True,
                                     stop=not f_bkv)
                    if f_bkv:
                        nc.tensor.matmul(op_, ones_row[0:1, :],
                                         vec_sb["bv"][0:1, :], start=False,
                                         stop=True)
                    oc = p_tmp.tile([P, HD], F32, name=f"oc{mt}", tag="t")
                    nc.vector.tensor_copy(out=oc, in_=op_)
                    vt = vtok[mt]
                    nc.vector.memset(vt, 1.0)
                    vt3 = vt.rearrange("p (h w) -> p h w", w=DA)
                    nc.vector.scalar_tensor_tensor(
                        out=vt3[:, :, 0:D],
                        in0=vp.rearrange("p (h w) -> p h w", w=D),
                        scalar=rstd2c[:, mt:mt + 1],
                        in1=oc.rearrange("p (h w) -> p h w", w=D),
                        op0=OP.mult, op1=OP.add)

                # q for block B last — attention(A) doesn't need it
                for g in range(4):
                    qk_group(q_sb, wq_sb, xb, aq_sb, bq_t, g, 1, rstd1_b, mr1n_b)

            # ============ PHASE 2: attention + pipelined FFN ============
            out1 = {}
            ln3t = {}
            parked = []
            with ExitStack() as es2:
                p_sc = es2.enter_context(
                    tc.tile_pool(name="p_sc", bufs=2, space="PSUM"))
                p_aug = es2.enter_context(
                    tc.tile_pool(name="p_aug", bufs=1, space="PSUM"))
                ps2 = es2.enter_context(
                    tc.tile_pool(name="ps2", bufs=2, space="PSUM"))
                p_e = es2.enter_context(tc.tile_pool(name="p_e", bufs=3))
                p_oT = es2.enter_context(tc.tile_pool(name="p_oT", bufs=8))
                p_zz = es2.enter_context(tc.tile_pool(name="p_zz", bufs=2))
                p_sq3 = es2.enter_context(tc.tile_pool(name="p_sq3", bufs=8))
                p_bc3 = es2.enter_context(tc.tile_pool(name="p_bc3", bufs=1))
                p_w2 = es2.enter_context(tc.tile_pool(name="p_w2", bufs=10))
                p_rows3 = es2.enter_context(
                    tc.tile_pool(name="p_rows3", bufs=5))
                p_w1p = es2.enter_context(tc.tile_pool(name="p_w1p", bufs=1))
                w1p_sb = []
                for i in range(8):
                    t = p_w1p.tile([P, C], BF16, name=f"w1p{i}")
                    nc.gpsimd.dma_start(out=t, in_=w1p_d[i * P:(i + 1) * P, :])
                    w1p_sb.append(t)
                aug_cur = {}
                oT = {}

                def attn_group(nb, g, mt):
                    ns = slice(nb * 512, (nb + 1) * 512)
                    if mt == 0:
                        aug_cur[0] = p_aug.tile([P, 512], F32,
                                                name=f"a01_{nb}{g}", tag="a01")
                        aug_cur[1] = p_aug.tile([P, 512], F32,
                                                name=f"a23_{nb}{g}", tag="a23")
                    for pr in range(2):
                        s_ps = p_sc.tile([P, 1024], F32,
                                         name=f"s{nb}{g}{mt}{pr}", tag="sc")
                        for jj in range(2):
                            j = 2 * pr + jj
                            rs = slice(32 * j, 32 * (j + 1))
                            nc.tensor.matmul(
                                s_ps[:, jj * 512:(jj + 1) * 512],
                                k_sb[g][rs, mt * P:(mt + 1) * P],
                                q_sb[g][rs, ns], start=True, stop=True,
                                tile_position=(32 * j, 0))
                        et = p_e.tile([P, 1024], BF16,
                                      name=f"e{nb}{g}{mt}{pr}", tag="e")
                        nc.scalar.activation(out=et, in_=s_ps, func=AF.Exp)
                        for jj in range(2):
                            h = 4 * g + 2 * pr + jj
                            nc.tensor.matmul(
                                aug_cur[pr][64 * jj:64 * (jj + 1), :],
                                vtok[mt][:, h * DA:(h + 1) * DA],
                                et[:, jj * 512:(jj + 1) * 512],
                                start=(mt == 0), stop=(mt == KT - 1),
                                tile_position=(0, 64 * jj))

                def attn_drain(nb, g):
                    for pr in range(2):
                        zr = p_zz.tile([P, 512], F32, name=f"zr{nb}{g}{pr}",
                                       tag="zr", bufs=1)
                        nc.vector.memset(zr, 0.0)
                        nc.vector.tensor_copy(out=zr[32:33, :],
                                              in_=aug_cur[pr][32:33, :])
                        nc.vector.tensor_copy(out=zr[96:97, :],
                                              in_=aug_cur[pr][96:97, :])
                        zb = ps2.tile([P, 512], F32, name=f"zb{nb}{g}{pr}",
                                      tag="r")
                        nc.tensor.matmul(zb, e2, zr, start=True, stop=True)
                        zi = p_zz.tile([P, 512], F32, name=f"zi{nb}{g}{pr}",
                                       tag="zi")
                        nc.vector.reciprocal_approx_fast(out=zi, in_=zb)
                        o = p_oT.tile([P, 512], BF16, name=f"o{nb}{g}{pr}",
                                      tag="oT")
                        nc.vector.tensor_mul(out=o, in0=aug_cur[pr], in1=zi)
                        oT[(nb, 2 * g + pr)] = o

                def w1_ct(nb, ct):
                    ns = slice(nb * 512, (nb + 1) * 512)
                    u_ps = ps2.tile([P, 512], F32, name=f"u{nb}{ct}", tag="r")
                    for i in range(8):
                        nc.tensor.matmul(u_ps, w1p_sb[i][:, ct * P:(ct + 1) * P],
                                         oT[(nb, i)],
                                         start=(i == 0), stop=(i == 7))
                    t = p_tmp.tile([P, 512], F32, name=f"x{nb}{ct}", tag="t")
                    nc.vector.tensor_mul(out=t, in0=xb[ct][:, ns],
                                         in1=rstd1_b[:, ns])
                    t2 = p_tmp.tile([P, 512], F32, name=f"x2{nb}{ct}", tag="t")
                    nc.vector.tensor_tensor(out=t2, in0=t, in1=mr1n_b[:, ns],
                                            op=OP.add)
                    if f_a1:
                        nc.vector.tensor_scalar(
                            out=t2, in0=t2, scalar1=vec_sb["g1"][ct],
                            scalar2=vec_sb["be1"][ct], op0=OP.mult, op1=OP.add)
                    o1 = p_out1.tile([P, 512], BF16, name=f"o1_{nb}{ct}",
                                     tag="o1")
                    b1s = vec_sb["b1"][ct] if f_b1 else 0.0
                    nc.vector.scalar_tensor_tensor(
                        out=o1, in0=u_ps, scalar=b1s, in1=t2,
                        op0=OP.add, op1=OP.add)
                    out1[(nb, ct)] = o1
                    sq3 = p_sq3.tile([P, 512], BF16, name=f"sq3_{nb}{ct}",
                                     tag="s3")
                    nc.vector.tensor_mul(out=sq3, in0=o1, in1=o1)
                    return sq3

                def ln3_block(nb, sq3s):
                    srcs = [out1[(nb, ct)] for ct in range(KT)]
                    rstd3_r, mr3n_r = ln_rows(srcs, sq3s, 512, f"l3{nb}",
                                              ps2, "r", p_rows3)
                    rstd3_b = p_bc3.tile([P, 512], F32, name=f"r3b{nb}",
                                         tag="r3")
                    mr3n_b = p_bc3.tile([P, 512], F32, name=f"m3b{nb}",
                                        tag="m3")
                    bcast(rstd3_r, rstd3_b, 512, ps2, "r", f"b3{nb}")
                    bcast(mr3n_r, mr3n_b, 512, ps2, "r", f"c3{nb}")
                    for ct in range(KT):
                        t = p_tmp.tile([P, 512], F32, name=f"l3t{nb}{ct}",
                                       tag="t")
                        nc.vector.tensor_mul(out=t, in0=out1[(nb, ct)],
                                             in1=rstd3_b)
                        l3 = p_ln3.tile([P, 512], BF16, name=f"l3_{nb}{ct}",
                                        tag="l3")
                        nc.vector.tensor_tensor(out=l3, in0=t, in1=mr3n_b,
                                                op=OP.add)
                        ln3t[(nb, ct)] = l3

                w2t_cur = []

                def w2_slot(nb, ft, park):
                    fq, fi = ft // 4, ft % 4
                    if fi == 0:
                        w2t_cur.clear()
                        for ct in range(KT):
                            t = p_w2.tile([P, 512], BF16,
                                          name=f"w2_{nb}{fq}{ct}", tag="w2")
                            eng = nc.sync if ct % 2 == 0 else nc.gpsimd
                            eng.dma_start(
                                out=t, in_=w2_d[ct * P:(ct + 1) * P,
                                               fq * 512:(fq + 1) * 512])
                            w2t_cur.append(t)
                    h_ps = ps2.tile([P, 512], F32, name=f"h{nb}{ft}", tag="r")
                    for ct in range(KT):
                        nc.tensor.matmul(h_ps,
                                         w2t_cur[ct][:, fi * P:(fi + 1) * P],
                                         ln3t[(nb, ct)],
                                         start=(ct == 0), stop=(ct == KT - 1))
                    hp = p_park.tile([P, 512], BF16, name=f"hp{nb}{ft}",
                                     tag="pk")
                    if park:
                        nc.vector.tensor_copy(out=hp, in_=h_ps)
                    else:
                        b2s = vec_sb["b2"][ft] if f_b2 else 0.0
                        nc.scalar.activation(out=hp, in_=h_ps, func=AF.Gelu,
                                             bias=b2s)
                    return hp

                # ---- attention block A ----
                for g in range(4):
                    for mt in range(KT):
                        attn_group(0, g, mt)
                    attn_drain(0, g)

                # ---- W1(A) + LN3(A) woven with early attention(B) ----
                groups_b = [(g, mt) for g in range(4) for mt in range(KT)]
                bi = 0

                def feed_b(n_groups):
                    nonlocal bi
                    for _ in range(n_groups):
                        if bi < len(groups_b):
                            g, mt = groups_b[bi]
                            attn_group(1, g, mt)
                            if mt == KT - 1:
                                attn_drain(1, g)
                            bi += 1

                sq3a = []
                for ct in range(KT):
                    sq3a.append(w1_ct(0, ct))
                    feed_b(1)
                ln3_block(0, sq3a)
                feed_b(2)

                # ---- W2(A) (park, no gelu) interleaved with attention(B);
                # attention(B) finishes by slot ~12 so W1(B)/LN3(B) can
                # overlap the late W2(A) slots ----
                h1a = []
                for ft in range(16):
                    h1a.append(w2_slot(0, ft, park=True))
                    feed_b(2)
                feed_b(len(groups_b))  # flush any remainder
                sq3b = []
                for ct in range(KT):
                    sq3b.append(w1_ct(1, ct))
                    h1a.append(w2_slot(0, 16 + ct, park=True))
                ln3_block(1, sq3b)
                for ft in range(24, 32):
                    h1a.append(w2_slot(0, ft, park=True))

            # ---- gelu(A) + W3(A) ----
            with ExitStack() as es3:
                p_h2a = es3.enter_context(
                    tc.tile_pool(name="p_h2a", bufs=1, space="PSUM"))
                p_hg = es3.enter_context(tc.tile_pool(name="p_hg", bufs=6))
                p_w3 = es3.enter_context(tc.tile_pool(name="p_w3", bufs=3))
                p_fin = es3.enter_context(tc.tile_pool(name="p_fin", bufs=4))
                def w3_block(nb, hg_of_ft, h2_pool):
                    ns = slice(nb * 512, (nb + 1) * 512)
                    h2 = [h2_pool.tile([P, 512], F32, name=f"h2_{nb}{ct}",
                                       tag=f"h2{ct}") for ct in range(KT)]
                    for ft in range(32):
                        w3t = p_w3.tile([P, C], BF16, name=f"w3_{nb}{ft}",
                                        tag="w3")
                        eng = nc.sync if ft % 2 == 0 else nc.gpsimd
                        eng.dma_start(out=w3t,
                                      in_=w3_d[ft * P:(ft + 1) * P, :])
                        hg = hg_of_ft(ft)
                        for ct in range(KT):
                            nc.tensor.matmul(h2[ct],
                                             w3t[:, ct * P:(ct + 1) * P], hg,
                                             start=(ft == 0), stop=(ft == 31))
                    for ct in range(KT):
                        fin = p_fin.tile([P, 512], F32, name=f"f{nb}{ct}",
                                         tag="fin")
                        b3s = vec_sb["b3"][ct] if f_b3 else 0.0
                        nc.vector.scalar_tensor_tensor(
                            out=fin, in0=h2[ct], scalar=b3s,
                            in1=out1[(nb, ct)], op0=OP.add, op1=OP.add)
                        nc.sync.dma_start(out=OT[ct * P:(ct + 1) * P, ns],
                                          in_=fin)

                def gelu_a(ft):
                    hg = p_hg.tile([P, 512], BF16, name=f"hg{ft}", tag="hg")
                    b2s = vec_sb["b2"][ft] if f_b2 else 0.0
                    nc.scalar.activation(out=hg, in_=h1a[ft], func=AF.Gelu,
                                         bias=b2s)
                    return hg

                w3_block(0, gelu_a, p_h2a)

            # ---- W2(B) (gelu inline) + W3(B) ----
            with tc.tile_pool(name="ps4", bufs=2, space="PSUM") as ps4:
                hgb = []
                with tc.tile_pool(name="p_w2b", bufs=12) as p_w2b:
                    w2t_b = []

                    def w2_slot_b(ft):
                        fq, fi = ft // 4, ft % 4
                        if fi == 0:
                            w2t_b.clear()
                            for ct in range(KT):
                                t = p_w2b.tile([P, 512], BF16,
                                               name=f"w2b_{fq}{ct}", tag="w2b")
                                eng = nc.sync if ct % 2 == 0 else nc.gpsimd
                                eng.dma_start(
                                    out=t, in_=w2_d[ct * P:(ct + 1) * P,
                                                   fq * 512:(fq + 1) * 512])
                                w2t_b.append(t)
                        h_ps = ps4.tile([P, 512], F32, name=f"hb{ft}", tag="r")
                        for ct in range(KT):
                            nc.tensor.matmul(
                                h_ps, w2t_b[ct][:, fi * P:(fi + 1) * P],
                                ln3t[(1, ct)],
                                start=(ct == 0), stop=(ct == KT - 1))
                        hp = p_park.tile([P, 512], BF16, name=f"hpb{ft}",
                                         tag="pk")
                        b2s = vec_sb["b2"][ft] if f_b2 else 0.0
                        nc.scalar.activation(out=hp, in_=h_ps, func=AF.Gelu,
                                             bias=b2s)
                        return hp

                    for ft in range(32):
                        hgb.append(w2_slot_b(ft))

            with ExitStack() as es4:
                p_h2b = es4.enter_context(
                    tc.tile_pool(name="p_h2b", bufs=1, space="PSUM"))
                p_w3b = es4.enter_context(tc.tile_pool(name="p_w3b", bufs=3))
                p_finb = es4.enter_context(tc.tile_pool(name="p_finb", bufs=4))
                h2 = [p_h2b.tile([P, 512], F32, name=f"h2b{ct}",
                                 tag=f"h2b{ct}") for ct in range(KT)]
                ns = slice(512, 1024)
                for ft in range(32):
                    w3t = p_w3b.tile([P, C], BF16, name=f"w3b_{ft}", tag="w3b")
                    eng = nc.sync if ft % 2 == 0 else nc.gpsimd
                    eng.dma_start(out=w3t, in_=w3_d[ft * P:(ft + 1) * P, :])
                    for ct in range(KT):
                        nc.tensor.matmul(h2[ct], w3t[:, ct * P:(ct + 1) * P],
                                         hgb[ft],
                                         start=(ft == 0), stop=(ft == 31))
                for ct in range(KT):
                    fin = p_finb.tile([P, 512], F32, name=f"fb{ct}", tag="fin")
                    b3s = vec_sb["b3"][ct] if f_b3 else 0.0
                    nc.vector.scalar_tensor_tensor(
                        out=fin, in0=h2[ct], scalar=b3s, in1=out1[(1, ct)],
                        op0=OP.add, op1=OP.add)
                    nc.sync.dma_start(out=OT[ct * P:(ct + 1) * P, ns], in_=fin)

    nc.finalize()
    return nc


def _nontrivial(v, val):
    return not np.allclose(np.asarray(v), val, rtol=0.0, atol=0.0)


def kernel(x, y, Wq, Wk, Wv, W1, b1, g1, be1, g2, be2, g3, be3, W2, b2, W3, b3):
    x = np.asarray(x, np.float32)
    y = np.asarray(y, np.float32)
    bf = ml_dtypes.bfloat16
    g1 = np.asarray(g1, np.float32)
    be1 = np.asarray(be1, np.float32)
    g2 = np.asarray(g2, np.float32)
    be2 = np.asarray(be2, np.float32)
    g3 = np.asarray(g3, np.float32)
    be3 = np.asarray(be3, np.float32)

    f_a1 = _nontrivial(g1, 1.0) or _nontrivial(be1, 0.0)
    f_bq = _nontrivial(be1, 0.0)
    f_bkv = _nontrivial(be2, 0.0)
    f_b1 = _nontrivial(b1, 0.0)
    f_b2 = _nontrivial(b2, 0.0) or _nontrivial(be3, 0.0)
    f_b3 = _nontrivial(b3, 0.0)
    flags = (f_a1, f_bq, f_bkv, f_b1, f_b2, f_b3)

    if flags not in _BUILD_CACHE:
        _BUILD_CACHE[flags] = _build(flags)
    nc = _BUILD_CACHE[flags]

    # host-side weight prep (g folded in; attention scale folded into Wq)
    wq_e = (np.transpose(np.asarray(Wq, np.float32), (1, 0, 2)).reshape(C, HD)
            * g1[:, None] * (D ** -0.5))
    wk_e = (np.transpose(np.asarray(Wk, np.float32), (1, 0, 2)).reshape(C, HD)
            * g2[:, None])
    wv_e = (np.transpose(np.asarray(Wv, np.float32), (1, 0, 2)).reshape(C, HD)
            * g2[:, None])
    aq = wq_e.sum(axis=0).reshape(HD, 1)
    ak = wk_e.sum(axis=0).reshape(HD, 1)
    av = wv_e.sum(axis=0).reshape(1, HD).astype(np.float32)

    W1f = np.asarray(W1, np.float32)
    w1p = np.zeros((2 * HD, C), np.float32)
    for g in range(4):
        for pr in range(2):
            i = 2 * g + pr
            h0 = 4 * g + 2 * pr
            h1 = h0 + 1
            w1p[i * 128:i * 128 + 32] = W1f[h0 * 32:(h0 + 1) * 32]
            w1p[i * 128 + 64:i * 128 + 96] = W1f[h1 * 32:(h1 + 1) * 32]

    w2_e = np.asarray(W2, np.float32) * g3[:, None]
    b2e = np.asarray(b2, np.float32) + be3 @ np.asarray(W2, np.float32)

    m0 = {
        "wq": wq_e.astype(bf), "wk": wk_e.astype(bf), "wv": wv_e.astype(bf),
        "aq": np.ascontiguousarray(aq), "ak": np.ascontiguousarray(ak),
        "av": np.ascontiguousarray(av),
        "w1p": w1p.astype(bf), "w2": w2_e.astype(bf),
        "w3": np.asarray(W3, np.float32).astype(bf),
    }
    if f_bq:
        m0["bq"] = (be1 @ wq_e).reshape(HD, 1).astype(np.float32)
    if f_bkv:
        m0["bk"] = (be2 @ wk_e).reshape(HD, 1).astype(np.float32)
        m0["bv"] = (be2 @ wv_e).reshape(1, HD).astype(np.float32)
    if f_a1:
        m0["g1"] = g1.reshape(C, 1)
        m0["be1"] = be1.reshape(C, 1)
    if f_b1:
        m0["b1"] = np.asarray(b1, np.float32).reshape(C, 1)
    if f_b2:
        m0["b2"] = b2e.reshape(F, 1).astype(np.float32)
    if f_b3:
        m0["b3"] = np.asarray(b3, np.float32).reshape(C, 1)

    in_maps = []
    for b in range(NCORES):
        m = dict(m0)
        m["xb"] = np.ascontiguousarray(x[b].T).astype(bf)
        m["yb"] = np.ascontiguousarray(y[b].T).astype(bf)
        in_maps.append(m)

    global _LAST_IN_MAPS
    _LAST_IN_MAPS = in_maps
    res = bass_utils.run_bass_kernel_spmd(nc, in_maps, core_ids=list(range(NCORES)))
    out = np.stack([np.ascontiguousarray(r["OT"].T) for r in res.results])
    return out.astype(np.float32)
